# revision 13
# baseline (speedup 1.0000x reference)
"""GraphSAGE 3-layer GNN on 8 TRN2 NeuronCores.

Strategy (node-sharded, feature-replicated):
  - 50000 nodes padded to 50176 = 8 cores x 49 blocks x 128. Core c owns
    destination rows [6272c, 6272c+6272). The padded node space is further
    split into NPIECE=4 "pieces" (contiguous block ranges per core,
    concatenated across cores) so that (a) every gather index fits int16,
    (b) each piece is AllGather'd independently as soon as its blocks are
    done (overlapping the collective with compute), and (c) each
    supergroup's gathers split into 4 dma_gather calls that run
    concurrently on the 4 SWDGE queues (4 Q7 core pairs).
  - Per layer, every core gathers h[src] rows for its local edges with
    gpsimd.dma_gather (one row per edge, landing on one SBUF partition),
    then segment-sums them per 128-dst block with a one-hot (x 1/deg)
    matmul on TensorE:  meanT[feat,dst] += G[e,feat].T @ S[e,dst].
  - Dense SAGE terms run in transposed orientation so tanh bias is
    per-partition:  outT[feat_out, node] = Wl @ meanT + Wr @ hT (+bias).
  - Dropout masks are precomputed on the host as {0, 1.25} multipliers.
  - Layer 3 aggregates the W3l-projected features p = h2 @ W3l^T
    (128 dims), so only p is gathered; h2 stays local for the W3r term.

kernel(**inputs) takes full-size numpy inputs, does all index/layout prep on
the host, compiles one SPMD Bass program, runs it on cores 0-7, and returns
the full [50000, 128] float32 output.
"""

import math
from contextlib import ExitStack

import ml_dtypes
import numpy as np

import concourse.bass as bass
import concourse.bacc as bacc
import concourse.mybir as mybir
import concourse.tile as tile
from concourse.bass_utils import run_bass_kernel_spmd
from concourse.masks import make_identity

P = 128
NCORES = 8
DROP_P = 0.2
BF16 = ml_dtypes.bfloat16

F32 = mybir.dt.float32
BF16_T = mybir.dt.bfloat16
I16 = mybir.dt.int16


class Plan:
    pass


# ----------------------------------------------------------------------------
# Planning (shared across cores; the SPMD program structure depends on it)
# ----------------------------------------------------------------------------

def make_plan(n_nodes, n_edges, d_in, d_out, edge_index, gmax=16):
    pl = Plan()
    pl.N = n_nodes
    pl.E = n_edges
    pl.D = d_in
    pl.DO = d_out
    pl.KB = d_in // P
    assert d_in % P == 0 and d_out == P
    pl.NB = math.ceil(n_nodes / NCORES / P)     # blocks per core
    pl.NPC = pl.NB * P                          # padded nodes per core
    pl.NPAD = NCORES * pl.NPC

    # pieces: contiguous block ranges; each piece's global row count <= 32768
    npiece = min(4, pl.NB)
    while math.ceil(pl.NB / npiece) * P * NCORES > 32768:
        npiece += 1
    splits = np.array_split(np.arange(pl.NB), npiece)
    pl.NPIECE = npiece
    pl.piece_first = [int(s[0]) for s in splits]          # first block
    pl.piece_nb = [len(s) for s in splits]                # blocks in piece
    pl.piece_rows = [nb * P for nb in pl.piece_nb]        # rows per core
    pl.piece_grows = [NCORES * r for r in pl.piece_rows]  # global rows
    pl.gpb = np.concatenate([[0], np.cumsum(pl.piece_grows)]).astype(np.int64)
    pl.block_piece = np.zeros(pl.NB, dtype=np.int64)
    for q, s in enumerate(splits):
        pl.block_piece[s] = q

    src = np.asarray(edge_index[0], dtype=np.int64)
    dst = np.asarray(edge_index[1], dtype=np.int64)
    assert src.min() >= 0 and src.max() < n_nodes
    deg = np.bincount(dst, minlength=n_nodes)
    pl.invdeg = (1.0 / np.maximum(deg, 1)).astype(np.float32)

    def pad_id(n):
        c = n // pl.NPC
        o = n - c * pl.NPC
        lb = o // P
        q = pl.block_piece[lb]
        fb = np.asarray(pl.piece_first)[q]
        rows = np.asarray(pl.piece_rows)[q]
        return pl.gpb[q] + c * rows + (o - fb * P)

    pl.pad_of_node = pad_id(np.arange(n_nodes))

    core = dst // pl.NPC
    lb = (dst - core * pl.NPC) // P             # dst block within core
    pl.dst_local = (dst - core * pl.NPC) % P
    pl.core = core
    pl.lb = lb
    pl.src = src
    pl.dst = dst

    spad = pl.pad_of_node[src]
    pl.src_q = np.searchsorted(pl.gpb, spad, side="right") - 1
    pl.src_idx = (spad - pl.gpb[pl.src_q]).astype(np.int64)
    assert pl.src_idx.max() < 32768

    # chunk caps per (dst block, src piece): max need over cores
    key = (core * pl.NB + lb) * npiece + pl.src_q
    cnt = np.bincount(key, minlength=NCORES * pl.NB * npiece)
    cnt = cnt.reshape(NCORES, pl.NB, npiece)
    pl.nchunk_bq = -(-cnt.max(axis=0) // P)    # [NB, npiece]

    # supergroups: consecutive blocks, per-piece chunk sums <= gmax
    sgs = []
    cur, s_q = [], np.zeros(npiece, dtype=np.int64)
    for b in range(pl.NB):
        if cur and (s_q + pl.nchunk_bq[b] > gmax).any():
            sgs.append(cur)
            cur, s_q = [], np.zeros(npiece, dtype=np.int64)
        cur.append(b)
        s_q = s_q + pl.nchunk_bq[b]
    if cur:
        sgs.append(cur)

    # global chunk ordering: per sg -> per piece -> blocks in order
    pl.sgs = []
    cg = 0
    pl.block_chunks = [[] for _ in range(pl.NB)]  # (sg, piece, slot, cg)
    for si, blocks in enumerate(sgs):
        info = {"blocks": blocks, "q_start": [], "q_n": []}
        for q in range(npiece):
            start = cg
            for b in blocks:
                for _ in range(pl.nchunk_bq[b][q]):
                    pl.block_chunks[b].append((si, q, cg - start, cg))
                    cg += 1
            info["q_start"].append(start)
            info["q_n"].append(cg - start)
        pl.sgs.append(info)
    pl.NCHUNK = cg
    pl.GMAX = max(max(sg["q_n"]) for sg in pl.sgs)
    pl.SMAX = max(sum(sg["q_n"]) for sg in pl.sgs)
    return pl


# ----------------------------------------------------------------------------
# Per-core host packing
# ----------------------------------------------------------------------------

def _featT(arr, KB):
    """[n, KB*128] -> [128, KB, n]  (feature-major layout)."""
    n = arr.shape[0]
    return np.ascontiguousarray(arr.T.reshape(KB, P, n).transpose(1, 0, 2))


def pack_core(pl, c, x_padded, scale1, scale2):
    NB, NPC, NCHUNK, NQ = pl.NB, pl.NPC, pl.NCHUNK, pl.NPIECE
    mine = pl.core == c
    sidx, q, lb, dl = (pl.src_idx[mine], pl.src_q[mine],
                       pl.lb[mine], pl.dst_local[mine])
    w = pl.invdeg[pl.dst[mine]]

    order = np.lexsort((q, lb))
    sidx, q, lb, dl, w = (a[order] for a in (sidx, q, lb, dl, w))

    # chunk start (cg) per (block, piece)
    cg_start = np.zeros((NB, NQ), dtype=np.int64)
    for b in range(NB):
        for (si, qq, sl, cg) in pl.block_chunks[b]:
            if cg_start[b, qq] == 0 or sl == 0:
                pass
        firsts = {}
        for (si, qq, sl, cg) in pl.block_chunks[b]:
            if qq not in firsts:
                firsts[qq] = cg
        for qq, cg0 in firsts.items():
            cg_start[b, qq] = cg0

    key = lb * NQ + q
    grp_first = np.searchsorted(key, np.arange(NB * NQ))
    rank = np.arange(len(sidx)) - grp_first[key]
    assert (rank < pl.nchunk_bq[lb, q] * P).all(), "chunk overflow"
    pos = cg_start[lb, q] * P + rank

    idx_vals = np.zeros(NCHUNK * P, dtype=np.int16)
    idx_vals[pos] = sidx.astype(np.int16)

    S = np.zeros((P, NCHUNK, P), dtype=BF16)
    S[pos % P, pos // P, dl] = w.astype(BF16)

    # wrap idx into [128, NCHUNK*8] int16, per-gather-call 16-element wrap
    idx_all = np.zeros((P, NCHUNK * 8), dtype=np.int16)
    for sg in pl.sgs:
        for qq in range(NQ):
            s0, n = sg["q_start"][qq], sg["q_n"][qq]
            if n == 0:
                continue
            seg = idx_vals[s0 * P:(s0 + n) * P]
            blk = seg.reshape(-1, 16).T
            idx_all[:, s0 * 8:(s0 + n) * 8] = np.tile(blk, (8, 1))

    lo_node, hi_node = c * NPC, min((c + 1) * NPC, pl.N)
    nreal = hi_node - lo_node
    xo = np.zeros((NPC, pl.D), dtype=BF16)
    xo[:nreal] = x_padded[lo_node:hi_node]
    m1 = np.zeros((NPC, pl.D), dtype=np.float32)
    m2 = np.zeros((NPC, pl.D), dtype=np.float32)
    m1[:nreal] = scale1[lo_node:hi_node]
    m2[:nreal] = scale2[lo_node:hi_node]

    return {
        "s_mat": S,
        "idx_all": idx_all,
        "xT": _featT(xo, pl.KB).astype(BF16),
        "m1T": _featT(m1, pl.KB).astype(BF16),
        "m2T": _featT(m2, pl.KB).astype(BF16),
    }


# ----------------------------------------------------------------------------
# Bass program
# ----------------------------------------------------------------------------

def build_program(pl, n_cores=NCORES, g_bufs=3):
    D, DO, KB, NB, NPC, NPAD = pl.D, pl.DO, pl.KB, pl.NB, pl.NPC, pl.NPAD
    NCHUNK, GMAX, SMAX, NQ = pl.NCHUNK, pl.GMAX, pl.SMAX, pl.NPIECE
    Tanh = mybir.ActivationFunctionType.Tanh

    nc = bacc.Bacc("TRN2", target_bir_lowering=False, debug=False,
                   num_devices=n_cores, dynamic_dma_scratch_size=16384,
                   num_swdge_queues=4)

    # ---- DRAM parameters
    x_rows = nc.dram_tensor("x_rows", [NPAD, D], BF16_T, kind="ExternalInput")
    xT_d = nc.dram_tensor("xT", [P, KB, NPC], BF16_T, kind="ExternalInput")
    m1_d = nc.dram_tensor("m1T", [P, KB, NPC], BF16_T, kind="ExternalInput")
    m2_d = nc.dram_tensor("m2T", [P, KB, NPC], BF16_T, kind="ExternalInput")
    s_d = nc.dram_tensor("s_mat", [P, NCHUNK, P], BF16_T, kind="ExternalInput")
    idx_d = nc.dram_tensor("idx_all", [P, NCHUNK * 8], I16, kind="ExternalInput")
    w1l_d = nc.dram_tensor("w1l", [P, KB, D], BF16_T, kind="ExternalInput")
    w1r_d = nc.dram_tensor("w1r", [P, KB, D], BF16_T, kind="ExternalInput")
    w2l_d = nc.dram_tensor("w2l", [P, KB, D], BF16_T, kind="ExternalInput")
    w2r_d = nc.dram_tensor("w2r", [P, KB, D], BF16_T, kind="ExternalInput")
    w3l_d = nc.dram_tensor("w3l", [P, KB, DO], BF16_T, kind="ExternalInput")
    w3r_d = nc.dram_tensor("w3r", [P, KB, DO], BF16_T, kind="ExternalInput")
    bias_d = nc.dram_tensor("bias", [P, 2 * KB + 1], F32, kind="ExternalInput")
    out_d = nc.dram_tensor("out", [NPC, DO], F32, kind="ExternalOutput")

    rg = [list(range(n_cores))]

    with tile.TileContext(nc) as tc, ExitStack() as ctx:
        consts = ctx.enter_context(tc.tile_pool(name="consts", bufs=1))
        gpool = ctx.enter_context(tc.tile_pool(name="gpool", bufs=g_bufs))
        work = ctx.enter_context(tc.tile_pool(name="work", bufs=3))
        dram = ctx.enter_context(tc.tile_pool(name="dram", bufs=1, space="DRAM"))
        psA = ctx.enter_context(tc.tile_pool(name="psA", bufs=2, space="PSUM"))
        psB = ctx.enter_context(tc.tile_pool(name="psB", bufs=2, space="PSUM"))
        psT = ctx.enter_context(tc.tile_pool(name="psT", bufs=2, space="PSUM"))
        psP = ctx.enter_context(tc.tile_pool(name="psP", bufs=2, space="PSUM"))

        # ---- internal DRAM: per-piece collective bounce + gathered h
        cc1_q = [dram.tile([pl.piece_rows[q], D], BF16_T, name=f"cc1_{q}")
                 for q in range(NQ)]
        h1_q = [dram.tile([pl.piece_grows[q], D], BF16_T,
                          addr_space="Shared", name=f"h1_{q}")
                for q in range(NQ)]
        ccp_q = [dram.tile([pl.piece_rows[q], DO], BF16_T, name=f"ccp_{q}")
                 for q in range(NQ)]
        p_q = [dram.tile([pl.piece_grows[q], DO], BF16_T,
                         addr_space="Shared", name=f"p_{q}")
               for q in range(NQ)]

        # ---- resident SBUF
        idx_sb = consts.tile([P, NCHUNK * 8], I16)
        nc.sync.dma_start(idx_sb[:], idx_d[:])
        w1l = consts.tile([P, KB, D], BF16_T)
        nc.sync.dma_start(w1l[:], w1l_d[:])
        w1r = consts.tile([P, KB, D], BF16_T)
        nc.sync.dma_start(w1r[:], w1r_d[:])
        w2l = consts.tile([P, KB, D], BF16_T)
        nc.sync.dma_start(w2l[:], w2l_d[:])
        w2r = consts.tile([P, KB, D], BF16_T)
        nc.sync.dma_start(w2r[:], w2r_d[:])
        w3l = consts.tile([P, KB, DO], BF16_T)
        nc.sync.dma_start(w3l[:], w3l_d[:])
        w3r = consts.tile([P, KB, DO], BF16_T)
        nc.sync.dma_start(w3r[:], w3r_d[:])
        bias_sb = consts.tile([P, 2 * KB + 1], F32)
        nc.sync.dma_start(bias_sb[:], bias_d[:])
        ident_b = consts.tile([P, P], BF16_T)
        make_identity(nc, ident_b)
        ident_f = consts.tile([P, P], F32)
        make_identity(nc, ident_f)

        # per-block resident hT tiles (feature-major current features)
        ht = []
        for b in range(NB):
            t = consts.tile([P, KB, P], BF16_T, name=f"ht{b}", tag=f"ht{b}")
            nc.sync.dma_start(t[:], xT_d[:, :, b * P:(b + 1) * P])
            ht.append(t)

        x_views = [x_rows[int(pl.gpb[q]):int(pl.gpb[q + 1]), :]
                   for q in range(NQ)]
        layers = [
            dict(wl=w1l, wr=w1r, mask=m1_d, bcol=0,
                 src=x_views, elem=D),
            dict(wl=w2l, wr=w2r, mask=m2_d, bcol=KB,
                 src=[t[:] for t in h1_q], elem=D),
            dict(wr3=w3r, bcol=2 * KB, src=[t[:] for t in p_q], elem=DO),
        ]
        # piece -> last block index (for inline AllGather launch)
        piece_last = [pl.piece_first[q] + pl.piece_nb[q] - 1
                      for q in range(NQ)]

        for li, L in enumerate(layers):
            elem = L["elem"]
            last3 = li == 2

            for si, sg in enumerate(pl.sgs):
                # stream this supergroup's S chunks (contiguous cg range)
                sg_c0 = sg["q_start"][0]
                sg_nc = sum(sg["q_n"])
                s_t = gpool.tile([P, SMAX, P], BF16_T, tag="s")
                nc.scalar.dma_start(
                    s_t[:, :sg_nc, :], s_d[:, sg_c0:sg_c0 + sg_nc, :])
                # 4 gathers (one per piece) on the 4 SWDGE queues
                tiles = [None] * NQ
                for q in range(NQ):
                    n = sg["q_n"][q]
                    if n == 0:
                        continue
                    s0 = sg["q_start"][q]
                    g_t = gpool.tile([P, GMAX, D], BF16_T, tag=f"g{q}")
                    if last3:
                        gv = g_t.rearrange("p g (a b) -> p (g a) b", b=DO)
                    else:
                        gv = g_t
                    nc.gpsimd.dma_gather(
                        gv[:, :n, :elem], L["src"][q],
                        idx_sb[:, s0 * 8:(s0 + n) * 8],
                        n * P, n * P, elem, single_packet=False,
                        queue_num=q % 4)
                    tiles[q] = gv

                for b in sg["blocks"]:
                    bsl = slice(b * P, (b + 1) * P)
                    my = [(tiles[qq], sl, cg)
                          for (s, qq, sl, cg) in pl.block_chunks[b]
                          if s == si]
                    nch = len(my)

                    if not last3:
                        # segment mean (transposed): meanT[feat,dst]
                        mps = psA.tile([P, KB, P], F32, tag="acc")
                        for k in range(KB):
                            for ci, (gt, sl, cg) in enumerate(my):
                                nc.tensor.matmul(
                                    mps[:, k, :],
                                    gt[:, sl, k * P:(k + 1) * P],
                                    s_t[:, cg - sg_c0, :],
                                    start=(ci == 0), stop=(ci == nch - 1))
                        m_sb = work.tile([P, KB, P], BF16_T, tag="msb")
                        if nch == 0:
                            nc.vector.memset(m_sb[:], 0.0)
                        else:
                            nc.vector.tensor_copy(m_sb[:], mps[:])

                        # dense: outT[feat_out, node] = Wl@meanT + Wr@hT
                        ops = psB.tile([P, KB, P], F32, tag="out")
                        for bank in range(KB):
                            for k in range(KB):
                                nc.tensor.matmul(
                                    ops[:, bank, :],
                                    L["wl"][:, k, bank * P:(bank + 1) * P],
                                    m_sb[:, k, :],
                                    start=(k == 0), stop=False)
                            for k in range(KB):
                                nc.tensor.matmul(
                                    ops[:, bank, :],
                                    L["wr"][:, k, bank * P:(bank + 1) * P],
                                    ht[b][:, k, :],
                                    start=False, stop=(k == KB - 1))

                        # epilogue: tanh(+bias), dropout mask, update hT
                        mk_t = work.tile([P, KB, P], BF16_T, tag="mk")
                        nc.sync.dma_start(mk_t[:], L["mask"][:, :, bsl])
                        a_sb = work.tile([P, KB, P], BF16_T, tag="act")
                        for bank in range(KB):
                            nc.scalar.activation(
                                a_sb[:, bank, :], ops[:, bank, :], Tanh,
                                bias=bias_sb[:, L["bcol"] + bank:
                                             L["bcol"] + bank + 1])
                        nc.vector.tensor_mul(
                            out=ht[b][:], in0=a_sb[:], in1=mk_t[:])

                        q_of_b = int(pl.block_piece[b])
                        row0 = (b - pl.piece_first[q_of_b]) * P
                        if li == 0:
                            # node-major copy for AllGather input
                            nm = work.tile([P, D], BF16_T, tag="nm")
                            for bank in range(KB):
                                tp = psT.tile([P, P], BF16_T, tag="tp")
                                nc.tensor.transpose(
                                    tp, ht[b][:, bank, :], ident_b)
                                nc.vector.tensor_copy(
                                    nm[:, bank * P:(bank + 1) * P], tp)
                            nc.sync.dma_start(
                                cc1_q[q_of_b][row0:row0 + P, :], nm)

                        if li == 1:
                            # p = h2 @ W3l^T (node-major) for layer-3 gather
                            pp = psP.tile([P, DO], F32, tag="pp")
                            for k in range(KB):
                                nc.tensor.matmul(
                                    pp, ht[b][:, k, :], w3l[:, k, :],
                                    start=(k == 0), stop=(k == KB - 1))
                            p_sb = work.tile([P, DO], BF16_T, tag="pnm")
                            nc.vector.tensor_copy(p_sb, pp)
                            nc.sync.dma_start(
                                ccp_q[q_of_b][row0:row0 + P, :], p_sb)

                        # piece finished -> launch its AllGather now
                        if b == piece_last[q_of_b]:
                            if li == 0:
                                nc.gpsimd.collective_compute(
                                    "AllGather", mybir.AluOpType.bypass,
                                    replica_groups=rg,
                                    ins=[cc1_q[q_of_b].opt()],
                                    outs=[h1_q[q_of_b].opt()])
                            elif li == 1:
                                nc.gpsimd.collective_compute(
                                    "AllGather", mybir.AluOpType.bypass,
                                    replica_groups=rg,
                                    ins=[ccp_q[q_of_b].opt()],
                                    outs=[p_q[q_of_b].opt()])
                    else:
                        # layer 3: outT = mean(p)^T + W3r @ hT, tanh, output
                        ops = psB.tile([P, KB, P], F32, tag="out")
                        o3 = ops[:, 0, :]
                        for ci, (gt, sl, cg) in enumerate(my):
                            nc.tensor.matmul(
                                o3, gt[:, sl, :], s_t[:, cg - sg_c0, :],
                                start=(ci == 0), stop=False)
                        for k in range(KB):
                            nc.tensor.matmul(
                                o3, L["wr3"][:, k, :], ht[b][:, k, :],
                                start=(nch == 0 and k == 0),
                                stop=(k == KB - 1))
                        o_sb = work.tile([P, DO], F32, tag="o3")
                        nc.scalar.activation(
                            o_sb, o3, Tanh,
                            bias=bias_sb[:, L["bcol"]:L["bcol"] + 1])
                        tpf = psP.tile([P, DO], F32, tag="pp")
                        nc.tensor.transpose(tpf, o_sb, ident_f)
                        onm = work.tile([P, DO], F32, tag="onm")
                        nc.vector.tensor_copy(onm, tpf)
                        nc.sync.dma_start(out_d[bsl, :], onm)

    nc.compile()
    return nc


# ----------------------------------------------------------------------------
# Host driver
# ----------------------------------------------------------------------------

def prepare(x, edge_index, mask1, mask2,
            W1l, b1, W1r, W2l, b2, W2r, W3l, b3, W3r, gmax=16):
    N, D = x.shape
    DO = W3l.shape[0]
    E = edge_index.shape[1]
    pl = make_plan(N, E, D, DO, edge_index, gmax=gmax)
    KB = pl.KB

    x_bf = x.astype(BF16)
    # x_rows in padded (piece-permuted) order
    x_pad = np.zeros((pl.NPAD, D), dtype=BF16)
    x_pad[pl.pad_of_node] = x_bf
    scale1 = ((mask1 > DROP_P) / (1.0 - DROP_P)).astype(np.float32)
    scale2 = ((mask2 > DROP_P) / (1.0 - DROP_P)).astype(np.float32)

    def packw(W):
        return np.ascontiguousarray(
            W.T.reshape(KB, P, W.shape[0]).transpose(1, 0, 2)).astype(BF16)

    bias = np.zeros((P, 2 * KB + 1), dtype=np.float32)
    for k in range(KB):
        bias[:, k] = b1[k * P:(k + 1) * P]
        bias[:, KB + k] = b2[k * P:(k + 1) * P]
    bias[:, 2 * KB] = b3[:P]

    shared = {
        "x_rows": x_pad,
        "w1l": packw(W1l), "w1r": packw(W1r),
        "w2l": packw(W2l), "w2r": packw(W2r),
        "w3l": packw(W3l), "w3r": packw(W3r),
        "bias": bias,
    }
    in_maps = []
    for c in range(NCORES):
        m = dict(shared)
        m.update(pack_core(pl, c, x_bf, scale1, scale2))
        in_maps.append(m)
    return pl, in_maps


def kernel(x, edge_index, mask1, mask2,
           W1l, b1, W1r, W2l, b2, W2r, W3l, b3, W3r):
    x = np.asarray(x, dtype=np.float32)
    pl, in_maps = prepare(
        x, np.asarray(edge_index),
        np.asarray(mask1, dtype=np.float32),
        np.asarray(mask2, dtype=np.float32),
        np.asarray(W1l, np.float32), np.asarray(b1, np.float32),
        np.asarray(W1r, np.float32),
        np.asarray(W2l, np.float32), np.asarray(b2, np.float32),
        np.asarray(W2r, np.float32),
        np.asarray(W3l, np.float32), np.asarray(b3, np.float32),
        np.asarray(W3r, np.float32))
    nc = build_program(pl)
    res = run_bass_kernel_spmd(nc, in_maps, core_ids=list(range(NCORES)))
    N = x.shape[0]
    out = np.zeros((N, pl.DO), dtype=np.float32)
    for c in range(NCORES):
        lo, hi = c * pl.NPC, min((c + 1) * pl.NPC, N)
        out[lo:hi] = res.results[c]["out"][:hi - lo]
    return out


# revision 14
# speedup vs baseline: 1.0261x; 1.0261x over previous
"""GraphSAGE 3-layer GNN on 8 TRN2 NeuronCores.

Strategy (node-sharded, feature-replicated):
  - 50000 nodes padded to 50176 = 8 cores x 49 blocks x 128. Core c owns
    destination rows [6272c, 6272c+6272). The padded node space is further
    split into NPIECE=4 "pieces" (contiguous block ranges per core,
    concatenated across cores) so that (a) every gather index fits int16,
    (b) each piece is AllGather'd independently as soon as its blocks are
    done (overlapping the collective with compute), and (c) each
    supergroup's gathers split into 4 dma_gather calls that run
    concurrently on the 4 SWDGE queues (4 Q7 core pairs).
  - Per layer, every core gathers h[src] rows for its local edges with
    gpsimd.dma_gather (one row per edge, landing on one SBUF partition),
    then segment-sums them per 128-dst block with a one-hot (x 1/deg)
    matmul on TensorE:  meanT[feat,dst] += G[e,feat].T @ S[e,dst].
  - Dense SAGE terms run in transposed orientation so tanh bias is
    per-partition:  outT[feat_out, node] = Wl @ meanT + Wr @ hT (+bias).
  - Dropout masks are precomputed on the host as {0, 1.25} multipliers.
  - Layer 3 aggregates the W3l-projected features p = h2 @ W3l^T
    (128 dims), so only p is gathered; h2 stays local for the W3r term.

kernel(**inputs) takes full-size numpy inputs, does all index/layout prep on
the host, compiles one SPMD Bass program, runs it on cores 0-7, and returns
the full [50000, 128] float32 output.
"""

import math
from contextlib import ExitStack

import ml_dtypes
import numpy as np

import concourse.bass as bass
import concourse.bacc as bacc
import concourse.mybir as mybir
import concourse.tile as tile
from concourse.bass_utils import run_bass_kernel_spmd
from concourse.masks import make_identity

P = 128
NCORES = 8
DROP_P = 0.2
BF16 = ml_dtypes.bfloat16

F32 = mybir.dt.float32
BF16_T = mybir.dt.bfloat16
I16 = mybir.dt.int16


class Plan:
    pass


# ----------------------------------------------------------------------------
# Planning (shared across cores; the SPMD program structure depends on it)
# ----------------------------------------------------------------------------

def make_plan(n_nodes, n_edges, d_in, d_out, edge_index, gmax=16):
    pl = Plan()
    pl.N = n_nodes
    pl.E = n_edges
    pl.D = d_in
    pl.DO = d_out
    pl.KB = d_in // P
    assert d_in % P == 0 and d_out == P
    pl.NB = math.ceil(n_nodes / NCORES / P)     # blocks per core
    pl.NPC = pl.NB * P                          # padded nodes per core
    pl.NPAD = NCORES * pl.NPC

    # pieces: contiguous block ranges; each piece's global row count <= 32768
    npiece = min(2, pl.NB)
    while math.ceil(pl.NB / npiece) * P * NCORES > 32768:
        npiece += 1
    splits = np.array_split(np.arange(pl.NB), npiece)
    pl.NPIECE = npiece
    pl.piece_first = [int(s[0]) for s in splits]          # first block
    pl.piece_nb = [len(s) for s in splits]                # blocks in piece
    pl.piece_rows = [nb * P for nb in pl.piece_nb]        # rows per core
    pl.piece_grows = [NCORES * r for r in pl.piece_rows]  # global rows
    pl.gpb = np.concatenate([[0], np.cumsum(pl.piece_grows)]).astype(np.int64)
    pl.block_piece = np.zeros(pl.NB, dtype=np.int64)
    for q, s in enumerate(splits):
        pl.block_piece[s] = q

    src = np.asarray(edge_index[0], dtype=np.int64)
    dst = np.asarray(edge_index[1], dtype=np.int64)
    assert src.min() >= 0 and src.max() < n_nodes
    deg = np.bincount(dst, minlength=n_nodes)
    pl.invdeg = (1.0 / np.maximum(deg, 1)).astype(np.float32)

    def pad_id(n):
        c = n // pl.NPC
        o = n - c * pl.NPC
        lb = o // P
        q = pl.block_piece[lb]
        fb = np.asarray(pl.piece_first)[q]
        rows = np.asarray(pl.piece_rows)[q]
        return pl.gpb[q] + c * rows + (o - fb * P)

    pl.pad_of_node = pad_id(np.arange(n_nodes))

    core = dst // pl.NPC
    lb = (dst - core * pl.NPC) // P             # dst block within core
    pl.dst_local = (dst - core * pl.NPC) % P
    pl.core = core
    pl.lb = lb
    pl.src = src
    pl.dst = dst

    spad = pl.pad_of_node[src]
    pl.src_q = np.searchsorted(pl.gpb, spad, side="right") - 1
    pl.src_idx = (spad - pl.gpb[pl.src_q]).astype(np.int64)
    assert pl.src_idx.max() < 32768

    # chunk caps per (dst block, src piece): max need over cores
    key = (core * pl.NB + lb) * npiece + pl.src_q
    cnt = np.bincount(key, minlength=NCORES * pl.NB * npiece)
    cnt = cnt.reshape(NCORES, pl.NB, npiece)
    pl.nchunk_bq = -(-cnt.max(axis=0) // P)    # [NB, npiece]

    # supergroups: consecutive blocks, per-piece chunk sums <= gmax
    sgs = []
    cur, s_q = [], np.zeros(npiece, dtype=np.int64)
    for b in range(pl.NB):
        if cur and (s_q + pl.nchunk_bq[b] > gmax).any():
            sgs.append(cur)
            cur, s_q = [], np.zeros(npiece, dtype=np.int64)
        cur.append(b)
        s_q = s_q + pl.nchunk_bq[b]
    if cur:
        sgs.append(cur)

    # global chunk ordering: per sg -> per piece -> blocks in order
    pl.sgs = []
    cg = 0
    pl.block_chunks = [[] for _ in range(pl.NB)]  # (sg, piece, slot, cg)
    for si, blocks in enumerate(sgs):
        info = {"blocks": blocks, "q_start": [], "q_n": []}
        for q in range(npiece):
            start = cg
            for b in blocks:
                for _ in range(pl.nchunk_bq[b][q]):
                    pl.block_chunks[b].append((si, q, cg - start, cg))
                    cg += 1
            info["q_start"].append(start)
            info["q_n"].append(cg - start)
        pl.sgs.append(info)
    pl.NCHUNK = cg
    pl.GMAX = max(max(sg["q_n"]) for sg in pl.sgs)
    pl.SMAX = max(sum(sg["q_n"]) for sg in pl.sgs)
    return pl


# ----------------------------------------------------------------------------
# Per-core host packing
# ----------------------------------------------------------------------------

def _featT(arr, KB):
    """[n, KB*128] -> [128, KB, n]  (feature-major layout)."""
    n = arr.shape[0]
    return np.ascontiguousarray(arr.T.reshape(KB, P, n).transpose(1, 0, 2))


def pack_core(pl, c, x_padded, scale1, scale2):
    NB, NPC, NCHUNK, NQ = pl.NB, pl.NPC, pl.NCHUNK, pl.NPIECE
    mine = pl.core == c
    sidx, q, lb, dl = (pl.src_idx[mine], pl.src_q[mine],
                       pl.lb[mine], pl.dst_local[mine])
    w = pl.invdeg[pl.dst[mine]]

    order = np.lexsort((q, lb))
    sidx, q, lb, dl, w = (a[order] for a in (sidx, q, lb, dl, w))

    # chunk start (cg) per (block, piece)
    cg_start = np.zeros((NB, NQ), dtype=np.int64)
    for b in range(NB):
        for (si, qq, sl, cg) in pl.block_chunks[b]:
            if cg_start[b, qq] == 0 or sl == 0:
                pass
        firsts = {}
        for (si, qq, sl, cg) in pl.block_chunks[b]:
            if qq not in firsts:
                firsts[qq] = cg
        for qq, cg0 in firsts.items():
            cg_start[b, qq] = cg0

    key = lb * NQ + q
    grp_first = np.searchsorted(key, np.arange(NB * NQ))
    rank = np.arange(len(sidx)) - grp_first[key]
    assert (rank < pl.nchunk_bq[lb, q] * P).all(), "chunk overflow"
    pos = cg_start[lb, q] * P + rank

    idx_vals = np.zeros(NCHUNK * P, dtype=np.int16)
    idx_vals[pos] = sidx.astype(np.int16)

    S = np.zeros((P, NCHUNK, P), dtype=BF16)
    S[pos % P, pos // P, dl] = w.astype(BF16)

    # wrap idx into [128, NCHUNK*8] int16, per-gather-call 16-element wrap
    idx_all = np.zeros((P, NCHUNK * 8), dtype=np.int16)
    for sg in pl.sgs:
        for qq in range(NQ):
            s0, n = sg["q_start"][qq], sg["q_n"][qq]
            if n == 0:
                continue
            seg = idx_vals[s0 * P:(s0 + n) * P]
            blk = seg.reshape(-1, 16).T
            idx_all[:, s0 * 8:(s0 + n) * 8] = np.tile(blk, (8, 1))

    lo_node, hi_node = c * NPC, min((c + 1) * NPC, pl.N)
    nreal = hi_node - lo_node
    xo = np.zeros((NPC, pl.D), dtype=BF16)
    xo[:nreal] = x_padded[lo_node:hi_node]
    m1 = np.zeros((NPC, pl.D), dtype=np.float32)
    m2 = np.zeros((NPC, pl.D), dtype=np.float32)
    m1[:nreal] = scale1[lo_node:hi_node]
    m2[:nreal] = scale2[lo_node:hi_node]

    return {
        "s_mat": S,
        "idx_all": idx_all,
        "xT": _featT(xo, pl.KB).astype(BF16),
        "m1T": _featT(m1, pl.KB).astype(BF16),
        "m2T": _featT(m2, pl.KB).astype(BF16),
    }


# ----------------------------------------------------------------------------
# Bass program
# ----------------------------------------------------------------------------

def build_program(pl, n_cores=NCORES, g_bufs=3):
    D, DO, KB, NB, NPC, NPAD = pl.D, pl.DO, pl.KB, pl.NB, pl.NPC, pl.NPAD
    NCHUNK, GMAX, SMAX, NQ = pl.NCHUNK, pl.GMAX, pl.SMAX, pl.NPIECE
    Tanh = mybir.ActivationFunctionType.Tanh

    nc = bacc.Bacc("TRN2", target_bir_lowering=False, debug=False,
                   num_devices=n_cores, dynamic_dma_scratch_size=16384,
                   num_swdge_queues=4)

    # ---- DRAM parameters
    x_rows = nc.dram_tensor("x_rows", [NPAD, D], BF16_T, kind="ExternalInput")
    xT_d = nc.dram_tensor("xT", [P, KB, NPC], BF16_T, kind="ExternalInput")
    m1_d = nc.dram_tensor("m1T", [P, KB, NPC], BF16_T, kind="ExternalInput")
    m2_d = nc.dram_tensor("m2T", [P, KB, NPC], BF16_T, kind="ExternalInput")
    s_d = nc.dram_tensor("s_mat", [P, NCHUNK, P], BF16_T, kind="ExternalInput")
    idx_d = nc.dram_tensor("idx_all", [P, NCHUNK * 8], I16, kind="ExternalInput")
    w1l_d = nc.dram_tensor("w1l", [P, KB, D], BF16_T, kind="ExternalInput")
    w1r_d = nc.dram_tensor("w1r", [P, KB, D], BF16_T, kind="ExternalInput")
    w2l_d = nc.dram_tensor("w2l", [P, KB, D], BF16_T, kind="ExternalInput")
    w2r_d = nc.dram_tensor("w2r", [P, KB, D], BF16_T, kind="ExternalInput")
    w3l_d = nc.dram_tensor("w3l", [P, KB, DO], BF16_T, kind="ExternalInput")
    w3r_d = nc.dram_tensor("w3r", [P, KB, DO], BF16_T, kind="ExternalInput")
    bias_d = nc.dram_tensor("bias", [P, 2 * KB + 1], F32, kind="ExternalInput")
    out_d = nc.dram_tensor("out", [NPC, DO], F32, kind="ExternalOutput")

    rg = [list(range(n_cores))]

    with tile.TileContext(nc) as tc, ExitStack() as ctx:
        consts = ctx.enter_context(tc.tile_pool(name="consts", bufs=1))
        gpool = ctx.enter_context(tc.tile_pool(name="gpool", bufs=g_bufs))
        work = ctx.enter_context(tc.tile_pool(name="work", bufs=3))
        dram = ctx.enter_context(tc.tile_pool(name="dram", bufs=1, space="DRAM"))
        psA = ctx.enter_context(tc.tile_pool(name="psA", bufs=2, space="PSUM"))
        psB = ctx.enter_context(tc.tile_pool(name="psB", bufs=2, space="PSUM"))
        psT = ctx.enter_context(tc.tile_pool(name="psT", bufs=2, space="PSUM"))
        psP = ctx.enter_context(tc.tile_pool(name="psP", bufs=2, space="PSUM"))

        # ---- internal DRAM: per-piece collective bounce + gathered h
        cc1_q = [dram.tile([pl.piece_rows[q], D], BF16_T, name=f"cc1_{q}")
                 for q in range(NQ)]
        h1_q = [dram.tile([pl.piece_grows[q], D], BF16_T,
                          addr_space="Shared", name=f"h1_{q}")
                for q in range(NQ)]
        ccp_q = [dram.tile([pl.piece_rows[q], DO], BF16_T, name=f"ccp_{q}")
                 for q in range(NQ)]
        p_q = [dram.tile([pl.piece_grows[q], DO], BF16_T,
                         addr_space="Shared", name=f"p_{q}")
               for q in range(NQ)]

        # ---- resident SBUF
        idx_sb = consts.tile([P, NCHUNK * 8], I16)
        nc.sync.dma_start(idx_sb[:], idx_d[:])
        w1l = consts.tile([P, KB, D], BF16_T)
        nc.sync.dma_start(w1l[:], w1l_d[:])
        w1r = consts.tile([P, KB, D], BF16_T)
        nc.sync.dma_start(w1r[:], w1r_d[:])
        w2l = consts.tile([P, KB, D], BF16_T)
        nc.sync.dma_start(w2l[:], w2l_d[:])
        w2r = consts.tile([P, KB, D], BF16_T)
        nc.sync.dma_start(w2r[:], w2r_d[:])
        w3l = consts.tile([P, KB, DO], BF16_T)
        nc.sync.dma_start(w3l[:], w3l_d[:])
        w3r = consts.tile([P, KB, DO], BF16_T)
        nc.sync.dma_start(w3r[:], w3r_d[:])
        bias_sb = consts.tile([P, 2 * KB + 1], F32)
        nc.sync.dma_start(bias_sb[:], bias_d[:])
        ident_b = consts.tile([P, P], BF16_T)
        make_identity(nc, ident_b)
        ident_f = consts.tile([P, P], F32)
        make_identity(nc, ident_f)

        # per-block resident hT tiles (feature-major current features)
        ht = []
        for b in range(NB):
            t = consts.tile([P, KB, P], BF16_T, name=f"ht{b}", tag=f"ht{b}")
            nc.sync.dma_start(t[:], xT_d[:, :, b * P:(b + 1) * P])
            ht.append(t)

        x_views = [x_rows[int(pl.gpb[q]):int(pl.gpb[q + 1]), :]
                   for q in range(NQ)]
        layers = [
            dict(wl=w1l, wr=w1r, mask=m1_d, bcol=0,
                 src=x_views, elem=D),
            dict(wl=w2l, wr=w2r, mask=m2_d, bcol=KB,
                 src=[t[:] for t in h1_q], elem=D),
            dict(wr3=w3r, bcol=2 * KB, src=[t[:] for t in p_q], elem=DO),
        ]
        # piece -> last block index (for inline AllGather launch)
        piece_last = [pl.piece_first[q] + pl.piece_nb[q] - 1
                      for q in range(NQ)]

        for li, L in enumerate(layers):
            elem = L["elem"]
            last3 = li == 2

            for si, sg in enumerate(pl.sgs):
                # stream this supergroup's S chunks (contiguous cg range)
                sg_c0 = sg["q_start"][0]
                sg_nc = sum(sg["q_n"])
                s_t = gpool.tile([P, SMAX, P], BF16_T, tag="s")
                nc.scalar.dma_start(
                    s_t[:, :sg_nc, :], s_d[:, sg_c0:sg_c0 + sg_nc, :])
                # gathers: per piece, two sub-calls on separate queues
                tiles = {}
                sub_n1 = {}
                for q in range(NQ):
                    n = sg["q_n"][q]
                    if n == 0:
                        continue
                    s0 = sg["q_start"][q]
                    n1 = (n + 1) // 2
                    sub_n1[q] = n1
                    for sub, (o, m) in enumerate(((0, n1), (n1, n - n1))):
                        if m == 0:
                            continue
                        half_cap = (GMAX + 1) // 2
                        g_t = gpool.tile([P, half_cap, D], BF16_T,
                                         tag=f"g{q}_{sub}")
                        if last3:
                            gv = g_t.rearrange(
                                "p g (a b) -> p (g a) b", b=DO)
                        else:
                            gv = g_t
                        nc.gpsimd.dma_gather(
                            gv[:, :m, :elem], L["src"][q],
                            idx_sb[:, (s0 + o) * 8:(s0 + o + m) * 8],
                            m * P, m * P, elem, single_packet=False,
                            queue_num=(2 * q + sub) % 4)
                        tiles[(q, sub)] = gv

                for b in sg["blocks"]:
                    bsl = slice(b * P, (b + 1) * P)
                    my = []
                    for (s, qq, sl, cg) in pl.block_chunks[b]:
                        if s != si:
                            continue
                        n1 = sub_n1[qq]
                        if sl < n1:
                            my.append((tiles[(qq, 0)], sl, cg))
                        else:
                            my.append((tiles[(qq, 1)], sl - n1, cg))
                    nch = len(my)

                    if not last3:
                        # segment mean (transposed): meanT[feat,dst]
                        mps = psA.tile([P, KB, P], F32, tag="acc")
                        for k in range(KB):
                            for ci, (gt, sl, cg) in enumerate(my):
                                nc.tensor.matmul(
                                    mps[:, k, :],
                                    gt[:, sl, k * P:(k + 1) * P],
                                    s_t[:, cg - sg_c0, :],
                                    start=(ci == 0), stop=(ci == nch - 1))
                        m_sb = work.tile([P, KB, P], BF16_T, tag="msb")
                        if nch == 0:
                            nc.vector.memset(m_sb[:], 0.0)
                        else:
                            nc.vector.tensor_copy(m_sb[:], mps[:])

                        # dense: outT[feat_out, node] = Wl@meanT + Wr@hT
                        ops = psB.tile([P, KB, P], F32, tag="out")
                        for bank in range(KB):
                            for k in range(KB):
                                nc.tensor.matmul(
                                    ops[:, bank, :],
                                    L["wl"][:, k, bank * P:(bank + 1) * P],
                                    m_sb[:, k, :],
                                    start=(k == 0), stop=False)
                            for k in range(KB):
                                nc.tensor.matmul(
                                    ops[:, bank, :],
                                    L["wr"][:, k, bank * P:(bank + 1) * P],
                                    ht[b][:, k, :],
                                    start=False, stop=(k == KB - 1))

                        # epilogue: tanh(+bias), dropout mask, update hT
                        mk_t = work.tile([P, KB, P], BF16_T, tag="mk")
                        nc.sync.dma_start(mk_t[:], L["mask"][:, :, bsl])
                        a_sb = work.tile([P, KB, P], BF16_T, tag="act")
                        for bank in range(KB):
                            nc.scalar.activation(
                                a_sb[:, bank, :], ops[:, bank, :], Tanh,
                                bias=bias_sb[:, L["bcol"] + bank:
                                             L["bcol"] + bank + 1])
                        nc.vector.tensor_mul(
                            out=ht[b][:], in0=a_sb[:], in1=mk_t[:])

                        q_of_b = int(pl.block_piece[b])
                        row0 = (b - pl.piece_first[q_of_b]) * P
                        if li == 0:
                            # node-major copy for AllGather input
                            nm = work.tile([P, D], BF16_T, tag="nm")
                            for bank in range(KB):
                                tp = psT.tile([P, P], BF16_T, tag="tp")
                                nc.tensor.transpose(
                                    tp, ht[b][:, bank, :], ident_b)
                                nc.vector.tensor_copy(
                                    nm[:, bank * P:(bank + 1) * P], tp)
                            nc.sync.dma_start(
                                cc1_q[q_of_b][row0:row0 + P, :], nm)

                        if li == 1:
                            # p = h2 @ W3l^T (node-major) for layer-3 gather
                            pp = psP.tile([P, DO], F32, tag="pp")
                            for k in range(KB):
                                nc.tensor.matmul(
                                    pp, ht[b][:, k, :], w3l[:, k, :],
                                    start=(k == 0), stop=(k == KB - 1))
                            p_sb = work.tile([P, DO], BF16_T, tag="pnm")
                            nc.vector.tensor_copy(p_sb, pp)
                            nc.sync.dma_start(
                                ccp_q[q_of_b][row0:row0 + P, :], p_sb)

                        # piece finished -> launch its AllGather now
                        if b == piece_last[q_of_b]:
                            if li == 0:
                                nc.gpsimd.collective_compute(
                                    "AllGather", mybir.AluOpType.bypass,
                                    replica_groups=rg,
                                    ins=[cc1_q[q_of_b].opt()],
                                    outs=[h1_q[q_of_b].opt()])
                            elif li == 1:
                                nc.gpsimd.collective_compute(
                                    "AllGather", mybir.AluOpType.bypass,
                                    replica_groups=rg,
                                    ins=[ccp_q[q_of_b].opt()],
                                    outs=[p_q[q_of_b].opt()])
                    else:
                        # layer 3: outT = mean(p)^T + W3r @ hT, tanh, output
                        ops = psB.tile([P, KB, P], F32, tag="out")
                        o3 = ops[:, 0, :]
                        for ci, (gt, sl, cg) in enumerate(my):
                            nc.tensor.matmul(
                                o3, gt[:, sl, :], s_t[:, cg - sg_c0, :],
                                start=(ci == 0), stop=False)
                        for k in range(KB):
                            nc.tensor.matmul(
                                o3, L["wr3"][:, k, :], ht[b][:, k, :],
                                start=(nch == 0 and k == 0),
                                stop=(k == KB - 1))
                        o_sb = work.tile([P, DO], F32, tag="o3")
                        nc.scalar.activation(
                            o_sb, o3, Tanh,
                            bias=bias_sb[:, L["bcol"]:L["bcol"] + 1])
                        tpf = psP.tile([P, DO], F32, tag="pp")
                        nc.tensor.transpose(tpf, o_sb, ident_f)
                        onm = work.tile([P, DO], F32, tag="onm")
                        nc.vector.tensor_copy(onm, tpf)
                        nc.sync.dma_start(out_d[bsl, :], onm)

    nc.compile()
    return nc


# ----------------------------------------------------------------------------
# Host driver
# ----------------------------------------------------------------------------

def prepare(x, edge_index, mask1, mask2,
            W1l, b1, W1r, W2l, b2, W2r, W3l, b3, W3r, gmax=16):
    N, D = x.shape
    DO = W3l.shape[0]
    E = edge_index.shape[1]
    pl = make_plan(N, E, D, DO, edge_index, gmax=gmax)
    KB = pl.KB

    x_bf = x.astype(BF16)
    # x_rows in padded (piece-permuted) order
    x_pad = np.zeros((pl.NPAD, D), dtype=BF16)
    x_pad[pl.pad_of_node] = x_bf
    scale1 = ((mask1 > DROP_P) / (1.0 - DROP_P)).astype(np.float32)
    scale2 = ((mask2 > DROP_P) / (1.0 - DROP_P)).astype(np.float32)

    def packw(W):
        return np.ascontiguousarray(
            W.T.reshape(KB, P, W.shape[0]).transpose(1, 0, 2)).astype(BF16)

    bias = np.zeros((P, 2 * KB + 1), dtype=np.float32)
    for k in range(KB):
        bias[:, k] = b1[k * P:(k + 1) * P]
        bias[:, KB + k] = b2[k * P:(k + 1) * P]
    bias[:, 2 * KB] = b3[:P]

    shared = {
        "x_rows": x_pad,
        "w1l": packw(W1l), "w1r": packw(W1r),
        "w2l": packw(W2l), "w2r": packw(W2r),
        "w3l": packw(W3l), "w3r": packw(W3r),
        "bias": bias,
    }
    in_maps = []
    for c in range(NCORES):
        m = dict(shared)
        m.update(pack_core(pl, c, x_bf, scale1, scale2))
        in_maps.append(m)
    return pl, in_maps


def kernel(x, edge_index, mask1, mask2,
           W1l, b1, W1r, W2l, b2, W2r, W3l, b3, W3r):
    x = np.asarray(x, dtype=np.float32)
    pl, in_maps = prepare(
        x, np.asarray(edge_index),
        np.asarray(mask1, dtype=np.float32),
        np.asarray(mask2, dtype=np.float32),
        np.asarray(W1l, np.float32), np.asarray(b1, np.float32),
        np.asarray(W1r, np.float32),
        np.asarray(W2l, np.float32), np.asarray(b2, np.float32),
        np.asarray(W2r, np.float32),
        np.asarray(W3l, np.float32), np.asarray(b3, np.float32),
        np.asarray(W3r, np.float32))
    nc = build_program(pl)
    res = run_bass_kernel_spmd(nc, in_maps, core_ids=list(range(NCORES)))
    N = x.shape[0]
    out = np.zeros((N, pl.DO), dtype=np.float32)
    for c in range(NCORES):
        lo, hi = c * pl.NPC, min((c + 1) * pl.NPC, N)
        out[lo:hi] = res.results[c]["out"][:hi - lo]
    return out
